# revision 16
# baseline (speedup 1.0000x reference)
"""MoE (top-2 of 8 experts, B=8192, D=2048) on 8 Trainium2 NeuronCores.

Strategy (expert-parallel, per sharding hint): the host computes the gate
softmax + top-2 routing (float64 numpy; rank-2/3 margins are ~3e-5 so the
selection matches any f32 reference platform), dispatches each token's rows
to its experts' cores, and each core computes
    y_e = relu(x_e @ W[e].T + b[e]) * gate_scale
for its gathered tokens as an fp16 tiled matmul on the PE array.  The host
then scatter-adds the (at most 2) expert contributions per token.

v4 schedule notes (from NTFF profiles of v1-v3):
- Steady state is at the fp16 streaming bound (216 ns per [K128,N512]
  matmul; LDWEIGHTS hidden by the PE reorder window), so the wins are at
  the edges: start stall, HAM cold clock, and tail.
- The first-needed 2.5MB (wt[0] + xt[0]) is delivered in fine-grained
  chunks on the two fast HWDGE rings (SP ~150GB/s from ~8us, ACT
  ~165GB/s from ~11us) so the PE starts useful work at ~11us and crawls
  at DMA rate instead of idling; only ~10 garbage warmup matmuls are
  needed to bridge the preamble and keep the HAM clock un-throttled.
- wt[1..3] go on the slow-start gpsimd SWDGE ring + SP, where latency
  doesn't matter.
- y is fp16 (host upcasts).  Phase 2 runs m-outer and accumulates the
  n=1..3 epilogues into one [P, 1536] tile -> one 384KB DMA per m with
  3KB lines (1KB-line y tiles from v2/v3 made every ring packet-bound).
"""

import math

import numpy as np

B, D, E, TOP_K = 8192, 2048, 8, 2
N_CORES = 8
P = 128
KD = D // P  # 16 contraction chunks
NT = 4
NSZ = D // NT  # 512 output columns per psum tile
WARMUP_MM = 12

_F16 = np.float16

_nc_cache = {}


def _routing(x, Wg, bg):
    """Gate softmax + top-2 in float64; returns (idx [B,2] int, vals [B,2] f32)."""
    logits = x.astype(np.float64) @ Wg.astype(np.float64).T + bg.astype(np.float64)
    logits -= logits.max(-1, keepdims=True)
    eL = np.exp(logits)
    gate = eL / eL.sum(-1, keepdims=True)
    order = np.argsort(-gate, axis=-1, kind="stable")
    idx = order[:, :TOP_K]
    vals = np.take_along_axis(gate, idx, -1).astype(np.float32)
    return idx, vals


def _build(m_tiles):
    """Build + compile the per-core Bass kernel for C = m_tiles*128 tokens."""
    import concourse.mybir as mybir
    import concourse.tile as tile
    from concourse import bacc

    nc = bacc.Bacc("TRN2", target_bir_lowering=False)
    C = m_tiles * P
    xt = nc.dram_tensor("xt", [P, m_tiles, KD, P], mybir.dt.float16, kind="ExternalInput")
    wt = nc.dram_tensor("wt", [P, NT, KD, NSZ], mybir.dt.float16, kind="ExternalInput")
    bias = nc.dram_tensor("bias", [P, D], mybir.dt.float16, kind="ExternalInput")
    scale = nc.dram_tensor("scale", [P, m_tiles], mybir.dt.float32, kind="ExternalInput")
    y = nc.dram_tensor("y", [C, D], mybir.dt.float16, kind="ExternalOutput")

    with tile.TileContext(nc) as tc:
        with (
            tc.tile_pool(name="wp", bufs=1) as wp,
            tc.tile_pool(name="xp", bufs=1) as xp,
            tc.tile_pool(name="cp", bufs=1) as cp,
            tc.tile_pool(name="op", bufs=6) as op_,
            tc.tile_pool(name="oy", bufs=3) as oyp,
            tc.tile_pool(name="pp", bufs=8, space="PSUM") as pp,
        ):
            # Warm tile memset on DVE so warmup isn't gated behind any DMA.
            warm = cp.tile([P, 640], mybir.dt.float16, tag="warm", name="warm")
            nc.vector.memset(warm[:], 0.0)

            # Everything latency-critical rides the two HWDGE queues in
            # FIFO order; the gpsimd SWDGE queue stays empty until the y
            # writes (which are gated by epilogue deps) so it can never
            # starve the early loads on the shared DMA engines.
            xts = [None] * m_tiles

            wts = [None] * NT
            wts[0] = wp.tile([P, KD, NSZ], mybir.dt.float16, tag="wt0", name="wt_sb0")

            def load_xt_on(m, eng):
                t = xp.tile([P, KD, P], mybir.dt.float16, tag=f"xt{m}", name=f"xt_sb{m}")
                eng.dma_start(t[:], xt[:, m])
                xts[m] = t

            # Both HWDGE queues carry the start-critical pieces in
            # time-of-need order (FIFO per queue):
            #   ACT: xt0, wt0[kd12..15], xt1, xt2, xt4, xt6, ...
            #   SP:  wt0[kd0..7 in 4 chunks], wt0[kd8..11], bias, scale,
            #        xt3, xt5, xt7, ...
            load_xt_on(0, nc.scalar)
            nc.scalar.dma_start(wts[0][:, 12:16], wt[:, 0, 12:16])
            load_xt_on(1, nc.scalar)
            load_xt_on(2, nc.scalar)

            for c in range(4):
                nc.sync.dma_start(wts[0][:, 2 * c:2 * c + 2], wt[:, 0, 2 * c:2 * c + 2])
            nc.sync.dma_start(wts[0][:, 8:12], wt[:, 0, 8:12])
            bias_sb = cp.tile([P, D], mybir.dt.float16, tag="bias", name="bias_sb")
            scale_sb = cp.tile([P, m_tiles], mybir.dt.float32, tag="scale", name="scale_sb")
            for m in range(3, m_tiles):
                load_xt_on(m, nc.sync if m % 2 == 1 else nc.scalar)
                if m == 3:
                    # bias/scale aren't needed until the first epilogue
                    # (~21us); keep them out of the critical wt0/xt window.
                    nc.sync.dma_start(bias_sb[:], bias[:])
                    nc.sync.dma_start(scale_sb[:], scale[:])
            for n in range(1, NT):
                wts[n] = wp.tile([P, KD, NSZ], mybir.dt.float16, tag=f"wt{n}", name=f"wt_sb{n}")

            # wt1..3 chunk list: 12 x 0.5MB, emitted on the gpsimd engine
            # interleaved with the dep-gated phase-1 y triggers, which paces
            # the SWDGE queue so it never starves the xt stream.
            _wt_chunks = [(n, c) for n in range(1, NT) for c in range(4)]

            def load_wt_chunk(k, gate=None):
                n, c = _wt_chunks[k]
                sl = slice(c * (KD // 4), (c + 1) * (KD // 4))
                if gate is not None:
                    # Pin: a 1-row copy from the (dep-gated) epilogue output
                    # into the chunk creates a WAW dependency the scheduler
                    # cannot hoist the DMA past -- this paces the 6MB of
                    # wt[1..3] behind phase-1 progress so it never floods the
                    # DMA engines during the latency-critical start window.
                    nc.vector.tensor_copy(wts[n][0:1, sl.start, 0:NSZ], gate[0:1, 0:NSZ])
                nc.gpsimd.dma_start(wts[n][:, sl], wt[:, n, sl])

            # PE warmup: bridge the engine preamble until the first wt[0]
            # chunk lands (~11us), keeping the HAM clock busy.
            wps = pp.tile([P, NSZ], mybir.dt.float32, tag="ps", name="warmps")
            for _w in range(WARMUP_MM):
                nc.tensor.matmul(wps[:], warm[:, 0:P], warm[:, P:P + NSZ],
                                 start=True, stop=True)

            def epilogue_n0(ps, m):
                ot = op_.tile([P, NSZ], mybir.dt.float32, tag="ot", name="ot")
                nc.vector.tensor_tensor(
                    ot[:], ps[:], bias_sb[:, 0:NSZ], mybir.AluOpType.add
                )
                ot16 = op_.tile([P, NSZ], mybir.dt.float16, tag="ot16", name="ot16")
                nc.vector.tensor_scalar(
                    ot16[:], ot[:], scale_sb[:, m:m + 1], 0.0,
                    mybir.AluOpType.mult, mybir.AluOpType.max,
                )
                nc.gpsimd.dma_start(y[m * P:(m + 1) * P, 0:NSZ], ot16[:])
                return ot16

            # Phase 1: n=0 sweep over all m-tiles.
            for m in range(m_tiles):
                ps = pp.tile([P, NSZ], mybir.dt.float32, tag="ps", name="ps")
                for kd in range(KD):
                    nc.tensor.matmul(
                        ps[:], xts[m][:, kd], wts[0][:, kd],
                        start=(kd == 0), stop=(kd == KD - 1),
                    )
                ot16 = epilogue_n0(ps, m)
                if m < len(_wt_chunks):
                    load_wt_chunk(m, gate=ot16)
            for k in range(m_tiles, len(_wt_chunks)):
                load_wt_chunk(k)

            # Phase 2: m-outer / n-inner; 3 psum banks per m; epilogues
            # accumulate into one [P, 3*NSZ] fp16 tile -> single 384KB DMA
            # with 3KB lines.
            def epi_small(ps, m, n, eng):
                ot = op_.tile([P, NSZ], mybir.dt.float32, tag="ot", name="ot")
                nc.vector.tensor_tensor(
                    ot[:], ps[:], bias_sb[:, n * NSZ:(n + 1) * NSZ],
                    mybir.AluOpType.add
                )
                ot16 = op_.tile([P, NSZ], mybir.dt.float16, tag="ot16", name="ot16")
                nc.vector.tensor_scalar(
                    ot16[:], ot[:], scale_sb[:, m:m + 1], 0.0,
                    mybir.AluOpType.mult, mybir.AluOpType.max,
                )
                eng.dma_start(y[m * P:(m + 1) * P, n * NSZ:(n + 1) * NSZ], ot16[:])

            for m in range(m_tiles - 1):
                pss = [pp.tile([P, NSZ], mybir.dt.float32, tag="ps", name="ps")
                       for _ in range(NT - 1)]
                for kd in range(KD):
                    for j in range(NT - 1):
                        nc.tensor.matmul(
                            pss[j][:], xts[m][:, kd], wts[j + 1][:, kd],
                            start=(kd == 0), stop=(kd == KD - 1),
                        )
                oty = oyp.tile([P, (NT - 1) * NSZ], mybir.dt.float16, tag="oty", name="oty")
                for j in range(NT - 1):
                    n = j + 1
                    ot = op_.tile([P, NSZ], mybir.dt.float32, tag="ot", name="ot")
                    nc.vector.tensor_tensor(
                        ot[:], pss[j][:], bias_sb[:, n * NSZ:(n + 1) * NSZ],
                        mybir.AluOpType.add
                    )
                    nc.vector.tensor_scalar(
                        oty[:, j * NSZ:(j + 1) * NSZ], ot[:], scale_sb[:, m:m + 1], 0.0,
                        mybir.AluOpType.mult, mybir.AluOpType.max,
                    )
                eng = nc.sync if m % 2 == 0 else nc.gpsimd
                eng.dma_start(y[m * P:(m + 1) * P, NSZ:D], oty[:])

            # Final m-tile: per-n sequential kd loops so each n-panel's
            # epilogue + small DMA overlaps the next panel's matmuls; the
            # tail after the very last matmul is a single epilogue + 128KB.
            m = m_tiles - 1
            for j in range(NT - 1):
                ps = pp.tile([P, NSZ], mybir.dt.float32, tag="ps", name="ps")
                for kd in range(KD):
                    nc.tensor.matmul(
                        ps[:], xts[m][:, kd], wts[j + 1][:, kd],
                        start=(kd == 0), stop=(kd == KD - 1),
                    )
                if j < NT - 2:
                    epi_small(ps, m, j + 1, nc.sync)
                else:
                    # Very last panel: 128-col slices so DVE and the small y
                    # DMAs pipeline; the post-matmul tail is one thin slice
                    # plus the fixed end barrier.
                    n = j + 1
                    for c in range(2):
                        cs = slice(c * (NSZ // 2), (c + 1) * (NSZ // 2))
                        ot = op_.tile([P, NSZ // 2], mybir.dt.float32, tag="otc", name="otc")
                        nc.vector.tensor_tensor(
                            ot[:], ps[:, cs], bias_sb[:, n * NSZ + cs.start:n * NSZ + cs.stop],
                            mybir.AluOpType.add
                        )
                        ot16 = op_.tile([P, NSZ // 2], mybir.dt.float16, tag="otc16", name="otc16")
                        nc.vector.tensor_scalar(
                            ot16[:], ot[:], scale_sb[:, m:m + 1], 0.0,
                            mybir.AluOpType.mult, mybir.AluOpType.max,
                        )
                        nc.sync.dma_start(
                            y[m * P:(m + 1) * P, n * NSZ + cs.start:n * NSZ + cs.stop],
                            ot16[:])

    nc.compile()
    return nc


def _get_nc(m_tiles):
    if m_tiles not in _nc_cache:
        _nc_cache[m_tiles] = _build(m_tiles)
    return _nc_cache[m_tiles]


def _prep_inputs(x, W, b, idx, vals):
    """Per-core input maps: blocked fp16 xT/wT layouts + bias/scale tiles."""
    in_maps = []
    token_lists = []
    counts = []
    for e in range(E):
        tok = np.where((idx == e).any(axis=1))[0]
        token_lists.append(tok)
        counts.append(len(tok))
    c_max = max(counts)
    m_tiles = max(1, math.ceil(c_max / P))
    C = m_tiles * P

    for e in range(E):
        tok = token_lists[e]
        cnt = len(tok)
        Xp = np.zeros((C, D), dtype=_F16)
        Xp[:cnt] = x[tok].astype(_F16)
        xt_np = np.ascontiguousarray(
            Xp.reshape(m_tiles, P, KD, P).transpose(3, 0, 2, 1)
        )
        wt_np = np.ascontiguousarray(
            W[e].astype(_F16).reshape(NT, NSZ, KD, P).transpose(3, 0, 2, 1)
        )
        bias_np = np.ascontiguousarray(np.broadcast_to(b[e], (P, D)).astype(_F16))
        s_tok = np.zeros(C, dtype=np.float32)
        for k in range(TOP_K):
            sel = idx[tok, k] == e
            s_tok[:cnt][sel] = vals[tok[sel], k]
        scale_np = np.ascontiguousarray(s_tok.reshape(m_tiles, P).T)
        in_maps.append({"xt": xt_np, "wt": wt_np, "bias": bias_np, "scale": scale_np})
    return in_maps, token_lists, counts, m_tiles


def kernel(x, W, b, Wg, bg):
    from concourse.bass_utils import run_bass_kernel_spmd

    x = np.asarray(x, dtype=np.float32)
    W = np.asarray(W, dtype=np.float32)
    b = np.asarray(b, dtype=np.float32)
    Wg = np.asarray(Wg, dtype=np.float32)
    bg = np.asarray(bg, dtype=np.float32)

    idx, vals = _routing(x, Wg, bg)
    in_maps, token_lists, counts, m_tiles = _prep_inputs(x, W, b, idx, vals)
    nc = _get_nc(m_tiles)
    res = run_bass_kernel_spmd(nc, in_maps, core_ids=list(range(N_CORES)))

    out = np.zeros((B, D), dtype=np.float32)
    for e in range(E):
        ye = res.results[e]["y"]
        out[token_lists[e]] += ye[:counts[e]].astype(np.float32)
    return out
